# revision 4
# baseline (speedup 1.0000x reference)
"""Trainium2 Bass kernel for nn_ExpertClassifierBank.

Computes, for pooled [B,K,D], expert weights [E,C,D], indices [K], log_scales [E]:
    x = l2norm(pooled, axis=-1)
    w = l2norm(weights[idx], axis=-1)
    out[b,k,c] = min(exp(log_scales[idx[k]]), 100) * dot(x[b,k], w[k,c])

Sharding: data-parallel over batch B across 8 NeuronCores (512 rows each);
the (gathered) expert weight bank is replicated.

Device algorithm per core (B_loc=512, K=8, D=1024=8x128, C=100):
  - host pre-transposes x to [k, d, j, b] tiles so the contraction dim d sits on
    SBUF partitions (PE contracts along partitions);
  - W branch: square (ACT) -> per-(k,c) sum-of-squares via selector matmuls into
    one PSUM bank -> reciprocal (DVE) -> sqrt with scale^2 folded in (ACT) ->
    broadcast over partitions via selector matmul (PE) -> wn = w * rw (DVE).
    scale = min(exp(ls),100) is computed on device and folded into wn.
  - X branch per k: DMA 2MB tile, square (ACT), row sum-of-squares via selector
    matmuls (PE, accumulated in PSUM), main matmuls wn^T @ x -> logits in PSUM
    [c=100, b=512]; f = 1/sqrt(ss) (DVE reciprocal + ACT sqrt), broadcast over
    the c-partitions via selector matmul, final out = logits * f (DVE) -> DMA.
  - float32r is used for the N=512 matmuls (full PE rate, fp32 storage).
"""

import os
import time

import numpy as np

import concourse.bass as bass
import concourse.mybir as mybir
import concourse.tile as tile
from concourse import bacc
from concourse.bass_utils import run_bass_kernel_spmd

N_CORES = 8
B, K, D, C, E = 4096, 8, 1024, 100, 16
BLOC = B // N_CORES  # 512
P = 128
DC = D // P  # 8 d-chunks
HALF = 4  # k-batch size for the x-norm pipeline

F32 = mybir.dt.float32
F32R = mybir.dt.float32r
AF = mybir.ActivationFunctionType

# dtype used for the big N=512 matmuls; float32r streams at full PE rate.
# Walrus requires every producer of an f32r matmul operand to emit f32r,
# so the feeding tiles (x, x2, wn, fx, selectors) are declared float32r.
USE_F32R = os.environ.get("KERNEL_MM_DTYPE", "f32r") == "f32r"
MMDT = F32R if USE_F32R else F32

_CACHE = {}

LAST_RESULT = None
LAST_WALL_NS = None


def _build():
    nc = bacc.Bacc(
        "TRN2", target_bir_lowering=False, debug=False, num_devices=N_CORES
    )

    xt = nc.dram_tensor("xt", [K, P, DC, BLOC], MMDT, kind="ExternalInput").ap()
    wt = nc.dram_tensor("wt", [P, K, DC, C], F32, kind="ExternalInput").ap()
    lsg = nc.dram_tensor("lsg", [K, 1], F32, kind="ExternalInput").ap()
    selk4 = nc.dram_tensor("selk4", [P, HALF, HALF], MMDT, kind="ExternalInput").ap()
    selk8 = nc.dram_tensor("selk8", [P, K, K], F32, kind="ExternalInput").ap()
    selc4 = nc.dram_tensor("selc4", [HALF, HALF, C], MMDT, kind="ExternalInput").ap()
    selp8 = nc.dram_tensor("selp8", [K, K, P], F32, kind="ExternalInput").ap()
    out = nc.dram_tensor("out", [K, C, BLOC], F32, kind="ExternalOutput").ap()

    with tile.TileContext(nc) as tc:
        with (
            tc.tile_pool(name="const", bufs=1) as cpool,
            tc.tile_pool(name="wn", bufs=1) as wnpool,
            tc.tile_pool(name="small", bufs=1) as spool,
        ):
            # ---- constant / weight loads ----
            w_sb = cpool.tile([P, K, DC, C], F32)
            nc.sync.dma_start(w_sb[:], wt[:])
            selk4_sb = cpool.tile([P, HALF, HALF], MMDT)
            nc.sync.dma_start(selk4_sb[:], selk4[:])
            selk8_sb = cpool.tile([P, K, K], F32)
            nc.sync.dma_start(selk8_sb[:], selk8[:])
            selc4_sb = cpool.tile([HALF, HALF, C], MMDT)
            nc.sync.dma_start(selc4_sb[:], selc4[:])
            selp8_sb = cpool.tile([K, K, P], F32)
            nc.sync.dma_start(selp8_sb[:], selp8[:])
            lsg_sb = cpool.tile([K, 1], F32)
            nc.sync.dma_start(lsg_sb[:], lsg[:])

            # ---- per-expert scale: s2 = min(exp(ls), 100)^2 ----
            s_sb = spool.tile([K, 1], F32)
            nc.scalar.activation(s_sb[:], lsg_sb[:], AF.Exp)
            nc.vector.tensor_scalar_min(s_sb[:], s_sb[:], 100.0)
            s2_sb = spool.tile([K, 1], F32)
            nc.vector.tensor_mul(s2_sb[:], s_sb[:], s_sb[:])

            # ---- W normalization ----
            wn_sb = wnpool.tile([P, K, DC, C], MMDT)
            with (
                tc.tile_pool(name="w2", bufs=1) as w2pool,
                tc.tile_pool(name="pwss", bufs=1, space="PSUM") as pwss,
                tc.tile_pool(name="prwb", bufs=2, space="PSUM") as prwb,
            ):
                w2_sb = w2pool.tile([P, K, DC, C], F32)
                for k in range(K):
                    nc.scalar.activation(w2_sb[:, k], w_sb[:, k], AF.Square)
                wss = pwss.tile([K, C], F32)
                for k in range(K):
                    for j in range(DC):
                        nc.tensor.matmul(
                            wss[:],
                            lhsT=selk8_sb[:, k, :],
                            rhs=w2_sb[:, k, j, :],
                            start=(k == 0 and j == 0),
                            stop=(k == K - 1 and j == DC - 1),
                        )
                recw_sb = spool.tile([K, C], F32)
                nc.vector.reciprocal(recw_sb[:], wss[:])
                rw_sb = spool.tile([K, C], F32)
                # rw = sqrt(s2 / ||w||^2) = scale / ||w||
                nc.scalar.activation(
                    rw_sb[:], recw_sb[:], AF.Sqrt, scale=s2_sb[:]
                )
                for k in range(K):
                    rwb = prwb.tile([P, C], F32)
                    nc.tensor.matmul(
                        rwb[:], lhsT=selp8_sb[:, k, :], rhs=rw_sb[:],
                        start=True, stop=True,
                    )
                    nc.vector.tensor_mul(
                        wn_sb[:, k],
                        w_sb[:, k],
                        rwb[:, None, :].to_broadcast((P, DC, C)),
                    )

            # ---- X pipeline ----
            with (
                tc.tile_pool(name="x", bufs=3) as xpool,
                tc.tile_pool(name="x2", bufs=3) as x2pool,
                tc.tile_pool(name="osb", bufs=2) as opool,
                tc.tile_pool(name="fx", bufs=2) as fxpool,
                tc.tile_pool(name="pss", bufs=2, space="PSUM") as pss,
                tc.tile_pool(name="plog", bufs=4, space="PSUM") as plog,
                tc.tile_pool(name="pf", bufs=2, space="PSUM") as pf,
            ):
                for half in range(2):
                    ss = pss.tile([HALF, BLOC], F32)
                    logs = []
                    for i in range(HALF):
                        k = half * HALF + i
                        x_sb = xpool.tile([P, DC, BLOC], MMDT)
                        nc.sync.dma_start(x_sb[:], xt[k])
                        x2_sb = x2pool.tile([P, DC, BLOC], MMDT)
                        for j in range(DC):
                            nc.scalar.activation(
                                x2_sb[:, j], x_sb[:, j].bitcast(F32) if USE_F32R else x_sb[:, j], AF.Square
                            )
                        for j in range(DC):
                            nc.tensor.matmul(
                                ss[:],
                                lhsT=selk4_sb[:, i, :],
                                rhs=x2_sb[:, j],
                                start=(i == 0 and j == 0),
                                stop=(i == HALF - 1 and j == DC - 1),
                                skip_group_check=True,
                            )
                        lg = plog.tile([C, BLOC], F32)
                        for j in range(DC):
                            nc.tensor.matmul(
                                lg[:],
                                lhsT=wn_sb[:, k, j, :],
                                rhs=x_sb[:, j],
                                start=(j == 0),
                                stop=(j == DC - 1),
                                skip_group_check=True,
                            )
                        logs.append(lg)

                    recx_sb = fxpool.tile([HALF, BLOC], F32, tag="recx")
                    nc.vector.reciprocal(recx_sb[:], ss[:])
                    fx_sb = fxpool.tile([HALF, BLOC], MMDT, tag="fx")
                    nc.scalar.activation(fx_sb[:], recx_sb[:], AF.Sqrt)

                    for i in range(HALF):
                        k = half * HALF + i
                        fb = pf.tile([C, BLOC], F32)
                        nc.tensor.matmul(
                            fb[:],
                            lhsT=selc4_sb[:, i, :],
                            rhs=fx_sb[:],
                            start=True, stop=True,
                            skip_group_check=True,
                        )
                        lg_sb = opool.tile([C, BLOC], F32, tag="lgsb")
                        nc.scalar.activation(lg_sb[:], logs[i][:], AF.Copy)
                        o_sb = opool.tile([C, BLOC], F32, tag="osb")
                        nc.vector.tensor_mul(o_sb[:], lg_sb[:], fb[:])
                        nc.sync.dma_start(out[k], o_sb[:])

    nc.compile()
    return nc


def _host_prep(pooled, active_expert_indices, weights, log_scales):
    idx = np.asarray(active_expert_indices).astype(np.int64)
    pooled = np.asarray(pooled, dtype=np.float32)
    weights = np.asarray(weights, dtype=np.float32)
    log_scales = np.asarray(log_scales, dtype=np.float32)

    # x: [B,K,D] -> per-core [K, P, DC, BLOC]  (k, d, j, b)
    xt_all = np.ascontiguousarray(
        pooled.reshape(N_CORES, BLOC, K, DC, P).transpose(0, 2, 4, 3, 1)
    )
    # w: gather + [E->K,C,D] -> [P, K, DC, C]  (d, k, j, c)
    wg = weights[idx]
    wt = np.ascontiguousarray(
        wg.reshape(K, C, DC, P).transpose(3, 0, 2, 1)
    )
    lsg = np.ascontiguousarray(log_scales[idx].reshape(K, 1))

    selk4 = np.zeros((P, HALF, HALF), np.float32)
    for i in range(HALF):
        selk4[:, i, i] = 1.0
    selk8 = np.zeros((P, K, K), np.float32)
    for k in range(K):
        selk8[:, k, k] = 1.0
    selc4 = np.zeros((HALF, HALF, C), np.float32)
    for i in range(HALF):
        selc4[i, i, :] = 1.0
    selp8 = np.zeros((K, K, P), np.float32)
    for k in range(K):
        selp8[k, k, :] = 1.0

    shared = {
        "wt": wt, "lsg": lsg, "selk4": selk4, "selk8": selk8,
        "selc4": selc4, "selp8": selp8,
    }
    in_maps = [dict(shared, xt=np.ascontiguousarray(xt_all[co]))
               for co in range(N_CORES)]
    return in_maps


def kernel(pooled, active_expert_indices, weights, log_scales):
    global LAST_RESULT, LAST_WALL_NS
    if "nc" not in _CACHE:
        _CACHE["nc"] = _build()
    nc = _CACHE["nc"]

    in_maps = _host_prep(pooled, active_expert_indices, weights, log_scales)

    t0 = time.perf_counter_ns()
    res = run_bass_kernel_spmd(nc, in_maps, core_ids=list(range(N_CORES)))
    LAST_WALL_NS = time.perf_counter_ns() - t0
    LAST_RESULT = res

    # [co][K, C, BLOC] -> [B, K, C]
    full = np.stack([res.results[co]["out"] for co in range(N_CORES)])
    return np.ascontiguousarray(
        full.transpose(0, 3, 1, 2).reshape(B, K, C)
    ).astype(np.float32)


# revision 9
# speedup vs baseline: 31740.0987x; 31740.0987x over previous
"""Trainium2 Bass kernel for nn_ExpertClassifierBank.

Computes, for pooled [B,K,D], expert weights [E,C,D], indices [K], log_scales [E]:
    x = l2norm(pooled, axis=-1)
    w = l2norm(weights[idx], axis=-1)
    out[b,k,c] = min(exp(log_scales[idx[k]]), 100) * dot(x[b,k], w[k,c])

Sharding: data-parallel over batch B across 8 NeuronCores (512 rows each);
the gathered expert weight bank is replicated.

Device algorithm per core (B_loc=512, K=8, D=1024=8x128, C=100):
  - host pre-transposes x to [k, d, j, b] bf16 tiles so the contraction dim d
    sits on SBUF partitions (PE contracts along partitions); bf16 streams at
    1 cyc/row on the PE and halves HBM traffic.
  - W branch: square (DVE) -> per-(k,c) sum-of-squares via selector matmuls
    into one PSUM bank -> reciprocal (DVE) -> sqrt with scale^2 folded in
    (ACT) -> broadcast over partitions via selector matmul (PE) ->
    wn = w * rw (DVE, bf16 out). scale = min(exp(ls),100), computed on device.
  - X branch per k: DMA 1MB bf16 tile, square (split ACT/DVE), row
    sum-of-squares via selector matmuls accumulated in PSUM, main matmuls
    wn^T @ x -> logits PSUM [c=100, b=512]; f = 1/sqrt(ss) (DVE reciprocal +
    ACT sqrt, fp32 path), broadcast over the c-partitions via selector
    matmul (float32r), final out = logits * f (ACT copy + DVE mult) -> DMA.
  - The per-row normalizer f stays in fp32/f32r: it multiplies every output
    directly, so bf16 there would cost ~0.4% scale error.
"""

import time

import numpy as np
import ml_dtypes

import concourse.bass as bass
import concourse.mybir as mybir
import concourse.tile as tile
from concourse import bacc
from concourse.bass_utils import run_bass_kernel_spmd

N_CORES = 8
B, K, D, C, E = 4096, 8, 1024, 100, 16
BLOC = B // N_CORES  # 512
P = 128
DC = D // P  # 8 d-chunks
HALF = 4  # k-batch size for the x-norm pipeline
JH = DC // 2  # squares engine-split point

F32 = mybir.dt.float32
F32R = mybir.dt.float32r
BF16 = mybir.dt.bfloat16
AF = mybir.ActivationFunctionType
NPBF16 = ml_dtypes.bfloat16

_CACHE = {}

LAST_RESULT = None
LAST_WALL_NS = None


def _build():
    nc = bacc.Bacc(
        "TRN2", target_bir_lowering=False, debug=False, num_devices=N_CORES
    )

    xt = nc.dram_tensor("xt", [K, P, DC, BLOC], BF16, kind="ExternalInput").ap()
    wt = nc.dram_tensor("wt", [K, P, DC, C], BF16, kind="ExternalInput").ap()
    lsg = nc.dram_tensor("lsg", [K, 1], F32, kind="ExternalInput").ap()
    selk4 = nc.dram_tensor("selk4", [P, HALF, HALF], BF16, kind="ExternalInput").ap()
    selk8 = nc.dram_tensor("selk8", [P, K, K], BF16, kind="ExternalInput").ap()
    selc4 = nc.dram_tensor("selc4", [HALF, HALF, C], F32R, kind="ExternalInput").ap()
    selp8 = nc.dram_tensor("selp8", [K, K, P], F32, kind="ExternalInput").ap()
    out = nc.dram_tensor("out", [K, C, BLOC], F32, kind="ExternalOutput").ap()

    with tile.TileContext(nc) as tc:
        with (
            tc.tile_pool(name="const", bufs=1) as cpool,
            tc.tile_pool(name="xres", bufs=K) as xpool,
            tc.tile_pool(name="wres", bufs=K) as wpool,
            tc.tile_pool(name="wn", bufs=K) as wnpool,
            tc.tile_pool(name="small", bufs=1) as spool,
        ):
            # ---- input DMAs (x tiles stay resident all kernel) ----
            x_sbs = []
            for k in range(K):
                x_sb = xpool.tile([P, DC, BLOC], BF16, tag="x")
                nc.sync.dma_start(x_sb[:], xt[k])
                x_sbs.append(x_sb)

            w_sbs = []
            for k in range(K):
                w1 = wpool.tile([P, DC, C], BF16, tag="w")
                nc.sync.dma_start(w1[:], wt[k])
                w_sbs.append(w1)
            selk4_sb = cpool.tile([P, HALF, HALF], BF16)
            nc.sync.dma_start(selk4_sb[:], selk4[:])
            selk8_sb = cpool.tile([P, K, K], BF16)
            nc.sync.dma_start(selk8_sb[:], selk8[:])
            selc4_sb = cpool.tile([HALF, HALF, C], F32R)
            nc.sync.dma_start(selc4_sb[:], selc4[:])
            selp8_sb = cpool.tile([K, K, P], F32)
            nc.sync.dma_start(selp8_sb[:], selp8[:])
            lsg_sb = cpool.tile([K, 1], F32)
            nc.sync.dma_start(lsg_sb[:], lsg[:])

            # ---- per-expert scale: s2 = min(exp(ls), 100)^2 ----
            s_sb = spool.tile([K, 1], F32)
            nc.scalar.activation(s_sb[:], lsg_sb[:], AF.Exp)
            nc.vector.tensor_scalar_min(s_sb[:], s_sb[:], 100.0)
            s2_sb = spool.tile([K, 1], F32)
            nc.vector.tensor_mul(s2_sb[:], s_sb[:], s_sb[:])

            # ---- W normalization (wn = scale * w / ||w||, bf16) ----
            wn_sbs = [
                wnpool.tile([P, DC, C], BF16, tag="wn", name=f"wn{k}")
                for k in range(K)
            ]
            with (
                tc.tile_pool(name="w2", bufs=2) as w2pool,
                tc.tile_pool(name="pwss", bufs=1, space="PSUM") as pwss,
                tc.tile_pool(name="prwb", bufs=1, space="PSUM") as prwb,
            ):
                wss = pwss.tile([K, C], F32)
                for k in range(K):
                    w2_sb = w2pool.tile([P, DC, C], BF16, tag="w2")
                    nc.vector.tensor_mul(w2_sb[:], w_sbs[k][:], w_sbs[k][:])
                    for j in range(DC):
                        nc.tensor.matmul(
                            wss[:],
                            lhsT=selk8_sb[:, k, :],
                            rhs=w2_sb[:, j],
                            start=(k == 0 and j == 0),
                            stop=(k == K - 1 and j == DC - 1),
                            skip_group_check=True,
                        )
                recw_sb = spool.tile([K, C], F32)
                nc.vector.reciprocal(recw_sb[:], wss[:])
                rw_sb = spool.tile([K, C], F32)
                # rw = sqrt(s2 / ||w||^2) = scale / ||w||
                nc.scalar.activation(rw_sb[:], recw_sb[:], AF.Sqrt, scale=s2_sb[:])
                for k in range(K):
                    rwb = prwb.tile([P, C], F32)
                    nc.tensor.matmul(
                        rwb[:], lhsT=selp8_sb[:, k, :], rhs=rw_sb[:],
                        start=True, stop=True, skip_group_check=True,
                    )
                    nc.vector.tensor_mul(
                        wn_sbs[k][:],
                        w_sbs[k][:],
                        rwb[:, None, :].to_broadcast((P, DC, C)),
                    )

            # ---- X pipeline ----
            with (
                tc.tile_pool(name="x2", bufs=3) as x2pool,
                tc.tile_pool(name="osb", bufs=2) as opool,
                tc.tile_pool(name="fx", bufs=2) as fxpool,
                tc.tile_pool(name="plog", bufs=4, space="PSUM") as plog,
                tc.tile_pool(name="pf", bufs=2, space="PSUM") as pf,
                tc.tile_pool(name="pss", bufs=2, space="PSUM") as pss,
            ):
                for half in range(2):
                    ss = pss.tile([HALF, BLOC], F32)
                    for i in range(HALF):
                        k = half * HALF + i
                        x_sb = x_sbs[k]
                        # squares: low j-half on ACT, high j-half on DVE
                        x2a = x2pool.tile([P, JH, BLOC], BF16, tag="x2a")
                        nc.scalar.activation(x2a[:], x_sb[:, :JH], AF.Square)
                        x2b = x2pool.tile([P, JH, BLOC], BF16, tag="x2b")
                        nc.vector.tensor_mul(
                            x2b[:], x_sb[:, JH:], x_sb[:, JH:]
                        )
                        for j in range(DC):
                            src = x2a[:, j] if j < JH else x2b[:, j - JH]
                            nc.tensor.matmul(
                                ss[:],
                                lhsT=selk4_sb[:, i, :],
                                rhs=src,
                                start=(i == 0 and j == 0),
                                stop=(i == HALF - 1 and j == DC - 1),
                                skip_group_check=True,
                            )
                    recx_sb = fxpool.tile([HALF, BLOC], F32, tag="recx")
                    nc.vector.reciprocal(recx_sb[:], ss[:])
                    fx_sb = fxpool.tile([HALF, BLOC], F32R, tag="fx")
                    nc.scalar.activation(fx_sb[:], recx_sb[:], AF.Sqrt)

                    for i in range(HALF):
                        k = half * HALF + i
                        x_sb = x_sbs[k]
                        lg = plog.tile([C, BLOC], F32)
                        for j in range(DC):
                            nc.tensor.matmul(
                                lg[:],
                                lhsT=wn_sbs[k][:, j, :],
                                rhs=x_sb[:, j],
                                start=(j == 0),
                                stop=(j == DC - 1),
                                skip_group_check=True,
                            )
                        fb = pf.tile([C, BLOC], F32)
                        nc.tensor.matmul(
                            fb[:],
                            lhsT=selc4_sb[:, i, :],
                            rhs=fx_sb[:],
                            start=True, stop=True,
                            skip_group_check=True,
                        )
                        lg_sb = opool.tile([C, BLOC], F32, tag="lgsb")
                        nc.scalar.activation(lg_sb[:], lg[:], AF.Copy)
                        o_sb = opool.tile([C, BLOC], F32, tag="osb")
                        nc.vector.tensor_mul(o_sb[:], lg_sb[:], fb[:])
                        nc.sync.dma_start(out[k], o_sb[:])

    nc.compile()
    return nc


def _host_prep(pooled, active_expert_indices, weights, log_scales):
    idx = np.asarray(active_expert_indices).astype(np.int64)
    pooled = np.asarray(pooled, dtype=np.float32)
    weights = np.asarray(weights, dtype=np.float32)
    log_scales = np.asarray(log_scales, dtype=np.float32)

    # x: [B,K,D] -> bf16 -> per-core [K, P, DC, BLOC]  (k, d, j, b)
    pb = pooled.astype(NPBF16)
    xt_all = np.ascontiguousarray(
        pb.reshape(N_CORES, BLOC, K, DC, P).transpose(0, 2, 4, 3, 1)
    )
    # w: gather -> bf16 -> [K, P, DC, C]  (k, d, j, c)
    wg = weights[idx].astype(NPBF16)
    wt = np.ascontiguousarray(wg.reshape(K, C, DC, P).transpose(0, 3, 2, 1))
    lsg = np.ascontiguousarray(log_scales[idx].reshape(K, 1))

    selk4 = np.zeros((P, HALF, HALF), NPBF16)
    for i in range(HALF):
        selk4[:, i, i] = 1.0
    selk8 = np.zeros((P, K, K), NPBF16)
    for k in range(K):
        selk8[:, k, k] = 1.0
    selc4 = np.zeros((HALF, HALF, C), np.float32)
    for i in range(HALF):
        selc4[i, i, :] = 1.0
    selp8 = np.zeros((K, K, P), np.float32)
    for k in range(K):
        selp8[k, k, :] = 1.0

    shared = {
        "wt": wt, "lsg": lsg, "selk4": selk4, "selk8": selk8,
        "selc4": selc4, "selp8": selp8,
    }
    return [dict(shared, xt=np.ascontiguousarray(xt_all[co]))
            for co in range(N_CORES)]


def kernel(pooled, active_expert_indices, weights, log_scales):
    global LAST_RESULT, LAST_WALL_NS
    if "nc" not in _CACHE:
        _CACHE["nc"] = _build()
    nc = _CACHE["nc"]

    in_maps = _host_prep(pooled, active_expert_indices, weights, log_scales)

    t0 = time.perf_counter_ns()
    res = run_bass_kernel_spmd(nc, in_maps, core_ids=list(range(N_CORES)))
    LAST_WALL_NS = time.perf_counter_ns() - t0
    LAST_RESULT = res

    full = np.stack([res.results[co]["out"] for co in range(N_CORES)])
    return np.ascontiguousarray(
        full.transpose(0, 3, 1, 2).reshape(B, K, C)
    ).astype(np.float32)


# revision 13
# speedup vs baseline: 41141.8301x; 1.2962x over previous
"""Trainium2 Bass kernel for nn_ExpertClassifierBank.

Computes, for pooled [B,K,D], expert weights [E,C,D], indices [K], log_scales [E]:
    x = l2norm(pooled, axis=-1)
    w = l2norm(weights[idx], axis=-1)
    out[b,k,c] = min(exp(log_scales[idx[k]]), 100) * dot(x[b,k], w[k,c])

Sharding: data-parallel over batch B across 8 NeuronCores (512 rows each);
the gathered expert weight bank is replicated.

Device algorithm per core (B_loc=512, K=8, D=1024=8x128, C=100):
  - host pre-transposes x to [k, d, j, b] bf16 tiles so the contraction dim d
    sits on SBUF partitions (PE contracts along partitions); bf16 streams at
    1 cyc/row on the PE and halves HBM traffic.
  - W branch: square (DVE) -> per-(k,c) sum-of-squares via selector matmuls
    into one PSUM bank -> reciprocal (DVE) -> sqrt with scale^2 folded in
    (ACT) -> broadcast over partitions via selector matmul (PE) ->
    wn = w * rw (DVE, bf16 out). scale = min(exp(ls),100), computed on device.
  - X branch per k: DMA 1MB bf16 tile, square (split ACT/DVE), row
    sum-of-squares via selector matmuls accumulated in PSUM, main matmuls
    wn^T @ x -> logits PSUM [c=100, b=512]; f = 1/sqrt(ss) (DVE reciprocal +
    ACT sqrt, fp32 path), broadcast over the c-partitions via selector
    matmul (float32r), final out = logits * f (ACT copy + DVE mult) -> DMA.
  - The per-row normalizer f stays in fp32/f32r: it multiplies every output
    directly, so bf16 there would cost ~0.4% scale error.
"""

import time

import numpy as np
import ml_dtypes

import concourse.bass as bass
import concourse.mybir as mybir
import concourse.tile as tile
from concourse import bacc
from concourse.bass_utils import run_bass_kernel_spmd

N_CORES = 8
B, K, D, C, E = 4096, 8, 1024, 100, 16
BLOC = B // N_CORES  # 512
P = 128
DC = D // P  # 8 d-chunks
HALF = 4  # k-batch size for the x-norm pipeline
JH = DC // 2  # squares engine-split point

F32 = mybir.dt.float32
F32R = mybir.dt.float32r
BF16 = mybir.dt.bfloat16
AF = mybir.ActivationFunctionType
NPBF16 = ml_dtypes.bfloat16

_CACHE = {}

LAST_RESULT = None
LAST_WALL_NS = None


def _build():
    nc = bacc.Bacc(
        "TRN2", target_bir_lowering=False, debug=False, num_devices=N_CORES
    )

    xt = nc.dram_tensor("xt", [K, P, DC, BLOC], BF16, kind="ExternalInput").ap()
    wt = nc.dram_tensor("wt", [K, P, DC, C], BF16, kind="ExternalInput").ap()
    lsg = nc.dram_tensor("lsg", [K, 1], F32, kind="ExternalInput").ap()
    selk4 = nc.dram_tensor("selk4", [P, HALF, HALF], BF16, kind="ExternalInput").ap()
    selk8 = nc.dram_tensor("selk8", [P, K, K], BF16, kind="ExternalInput").ap()
    selc4 = nc.dram_tensor("selc4", [HALF, HALF, C], F32R, kind="ExternalInput").ap()
    selp8 = nc.dram_tensor("selp8", [K, K, P], F32, kind="ExternalInput").ap()
    out = nc.dram_tensor("out", [K, C, BLOC], F32, kind="ExternalOutput").ap()

    with tile.TileContext(nc) as tc:
        with (
            tc.tile_pool(name="const", bufs=1) as cpool,
            tc.tile_pool(name="xres", bufs=K) as xpool,
            tc.tile_pool(name="wres", bufs=K) as wpool,
            tc.tile_pool(name="wn", bufs=K) as wnpool,
            tc.tile_pool(name="small", bufs=1) as spool,
        ):
            # ---- input DMAs: small + W first (the W chain gates every
            # engine queue), then the 8MB of x tiles ----
            selk4_sb = cpool.tile([P, HALF, HALF], BF16)
            nc.sync.dma_start(selk4_sb[:], selk4[:])
            selk8_sb = cpool.tile([P, K, K], BF16)
            nc.sync.dma_start(selk8_sb[:], selk8[:])
            selc4_sb = cpool.tile([HALF, HALF, C], F32R)
            nc.sync.dma_start(selc4_sb[:], selc4[:])
            selp8_sb = cpool.tile([K, K, P], F32)
            nc.sync.dma_start(selp8_sb[:], selp8[:])
            lsg_sb = cpool.tile([K, 1], F32)
            nc.sync.dma_start(lsg_sb[:], lsg[:])

            w_sbs = []
            for k in range(K):
                w1 = wpool.tile([P, DC, C], BF16, tag="w", name=f"w{k}")
                nc.sync.dma_start(w1[:], wt[k])
                w_sbs.append(w1)
            x_sbs = []
            for k in range(K):
                x_sb = xpool.tile([P, DC, BLOC], BF16, tag="x", name=f"x{k}")
                nc.sync.dma_start(x_sb[:], xt[k])
                x_sbs.append(x_sb)

            # ---- per-expert scale: s2 = min(exp(ls), 100)^2 ----
            s_sb = spool.tile([K, 1], F32)
            nc.scalar.activation(s_sb[:], lsg_sb[:], AF.Exp)
            nc.vector.tensor_scalar_min(s_sb[:], s_sb[:], 100.0)
            s2_sb = spool.tile([K, 1], F32)
            nc.vector.tensor_mul(s2_sb[:], s_sb[:], s_sb[:])

            # ---- W normalization (wn = scale * w / ||w||, bf16) ----
            wn_sbs = [
                wnpool.tile([P, DC, C], BF16, tag="wn", name=f"wn{k}")
                for k in range(K)
            ]
            with (
                tc.tile_pool(name="w2", bufs=2) as w2pool,
                tc.tile_pool(name="pwss", bufs=1, space="PSUM") as pwss,
                tc.tile_pool(name="prwb", bufs=1, space="PSUM") as prwb,
            ):
                wss = pwss.tile([K, C], F32)
                for k in range(K):
                    w2_sb = w2pool.tile([P, DC, C], BF16, tag="w2")
                    nc.vector.tensor_mul(w2_sb[:], w_sbs[k][:], w_sbs[k][:])
                    for j in range(DC):
                        nc.tensor.matmul(
                            wss[:],
                            lhsT=selk8_sb[:, k, :],
                            rhs=w2_sb[:, j],
                            start=(k == 0 and j == 0),
                            stop=(k == K - 1 and j == DC - 1),
                            skip_group_check=True,
                        )
                recw_sb = spool.tile([K, C], F32)
                nc.vector.reciprocal(recw_sb[:], wss[:])
                rw_sb = spool.tile([K, C], F32)
                # rw = sqrt(s2 / ||w||^2) = scale / ||w||
                nc.scalar.activation(rw_sb[:], recw_sb[:], AF.Sqrt, scale=s2_sb[:])
                for k in range(K):
                    rwb = prwb.tile([P, C], F32)
                    nc.tensor.matmul(
                        rwb[:], lhsT=selp8_sb[:, k, :], rhs=rw_sb[:],
                        start=True, stop=True, skip_group_check=True,
                    )
                    nc.vector.tensor_mul(
                        wn_sbs[k][:],
                        w_sbs[k][:],
                        rwb[:, None, :].to_broadcast((P, DC, C)),
                    )

            # ---- X pipeline ----
            with (
                tc.tile_pool(name="x2", bufs=3) as x2pool,
                tc.tile_pool(name="osb", bufs=2) as opool,
                tc.tile_pool(name="fx", bufs=2) as fxpool,
                tc.tile_pool(name="plog", bufs=4, space="PSUM") as plog,
                tc.tile_pool(name="pf", bufs=2, space="PSUM") as pf,
                tc.tile_pool(name="pss", bufs=2, space="PSUM") as pss,
            ):
                for half in range(2):
                    ss = pss.tile([HALF, BLOC], F32)
                    for i in range(HALF):
                        k = half * HALF + i
                        x_sb = x_sbs[k]
                        # squares: low j-half on ACT, high j-half on DVE
                        x2a = x2pool.tile([P, JH, BLOC], BF16, tag="x2a")
                        nc.scalar.activation(x2a[:], x_sb[:, :JH], AF.Square)
                        x2b = x2pool.tile([P, JH, BLOC], BF16, tag="x2b")
                        nc.vector.tensor_mul(
                            x2b[:], x_sb[:, JH:], x_sb[:, JH:]
                        )
                        for j in range(DC):
                            src = x2a[:, j] if j < JH else x2b[:, j - JH]
                            nc.tensor.matmul(
                                ss[:],
                                lhsT=selk4_sb[:, i, :],
                                rhs=src,
                                start=(i == 0 and j == 0),
                                stop=(i == HALF - 1 and j == DC - 1),
                                skip_group_check=True,
                            )
                    recx_sb = fxpool.tile([HALF, BLOC], F32, tag="recx")
                    nc.vector.reciprocal(recx_sb[:], ss[:])
                    fx_sb = fxpool.tile([HALF, BLOC], F32R, tag="fx")
                    nc.scalar.activation(fx_sb[:], recx_sb[:], AF.Sqrt)

                    # all main matmuls first (no f dependency), then the
                    # f-broadcasts — keeps the in-order PE queue stall-free
                    lgs = []
                    for i in range(HALF):
                        k = half * HALF + i
                        x_sb = x_sbs[k]
                        lg = plog.tile([C, BLOC], F32, tag="lg", name=f"lg{k}")
                        for j in range(DC):
                            nc.tensor.matmul(
                                lg[:],
                                lhsT=wn_sbs[k][:, j, :],
                                rhs=x_sb[:, j],
                                start=(j == 0),
                                stop=(j == DC - 1),
                                skip_group_check=True,
                            )
                        lgs.append(lg)
                    for i in range(HALF):
                        k = half * HALF + i
                        fb = pf.tile([C, BLOC], F32)
                        nc.tensor.matmul(
                            fb[:],
                            lhsT=selc4_sb[:, i, :],
                            rhs=fx_sb[:],
                            start=True, stop=True,
                            skip_group_check=True,
                        )
                        lg_sb = opool.tile([C, BLOC], F32, tag="lgsb")
                        nc.scalar.activation(lg_sb[:], lgs[i][:], AF.Copy)
                        o_sb = opool.tile([C, BLOC], F32, tag="osb")
                        nc.vector.tensor_mul(o_sb[:], lg_sb[:], fb[:])
                        nc.sync.dma_start(out[k], o_sb[:])

    nc.compile()
    return nc


def _host_prep(pooled, active_expert_indices, weights, log_scales):
    idx = np.asarray(active_expert_indices).astype(np.int64)
    pooled = np.asarray(pooled, dtype=np.float32)
    weights = np.asarray(weights, dtype=np.float32)
    log_scales = np.asarray(log_scales, dtype=np.float32)

    # x: [B,K,D] -> bf16 -> per-core [K, P, DC, BLOC]  (k, d, j, b)
    pb = pooled.astype(NPBF16)
    xt_all = np.ascontiguousarray(
        pb.reshape(N_CORES, BLOC, K, DC, P).transpose(0, 2, 4, 3, 1)
    )
    # w: gather -> bf16 -> [K, P, DC, C]  (k, d, j, c)
    wg = weights[idx].astype(NPBF16)
    wt = np.ascontiguousarray(wg.reshape(K, C, DC, P).transpose(0, 3, 2, 1))
    lsg = np.ascontiguousarray(log_scales[idx].reshape(K, 1))

    selk4 = np.zeros((P, HALF, HALF), NPBF16)
    for i in range(HALF):
        selk4[:, i, i] = 1.0
    selk8 = np.zeros((P, K, K), NPBF16)
    for k in range(K):
        selk8[:, k, k] = 1.0
    selc4 = np.zeros((HALF, HALF, C), np.float32)
    for i in range(HALF):
        selc4[i, i, :] = 1.0
    selp8 = np.zeros((K, K, P), np.float32)
    for k in range(K):
        selp8[k, k, :] = 1.0

    shared = {
        "wt": wt, "lsg": lsg, "selk4": selk4, "selk8": selk8,
        "selc4": selc4, "selp8": selp8,
    }
    return [dict(shared, xt=np.ascontiguousarray(xt_all[co]))
            for co in range(N_CORES)]


def kernel(pooled, active_expert_indices, weights, log_scales):
    global LAST_RESULT, LAST_WALL_NS
    if "nc" not in _CACHE:
        _CACHE["nc"] = _build()
    nc = _CACHE["nc"]

    in_maps = _host_prep(pooled, active_expert_indices, weights, log_scales)

    t0 = time.perf_counter_ns()
    res = run_bass_kernel_spmd(nc, in_maps, core_ids=list(range(N_CORES)))
    LAST_WALL_NS = time.perf_counter_ns() - t0
    LAST_RESULT = res

    full = np.stack([res.results[co]["out"] for co in range(N_CORES)])
    return np.ascontiguousarray(
        full.transpose(0, 3, 1, 2).reshape(B, K, C)
    ).astype(np.float32)


# revision 21
# speedup vs baseline: 43437.1757x; 1.0558x over previous
"""Trainium2 Bass kernel for nn_ExpertClassifierBank.

Computes, for pooled [B,K,D], expert weights [E,C,D], indices [K], log_scales [E]:
    x = l2norm(pooled, axis=-1)
    w = l2norm(weights[idx], axis=-1)
    out[b,k,c] = min(exp(log_scales[idx[k]]), 100) * dot(x[b,k], w[k,c])

Sharding: data-parallel over batch B across 8 NeuronCores (512 rows each);
the gathered expert weight bank is replicated.

Device algorithm per core (B_loc=512, K=8, D=1024=8x128, C=100):
  - host pre-transposes x and w to [k, d, j, *] bf16 tiles so the contraction
    dim d sits on SBUF partitions (PE contracts along partitions); bf16
    streams at 1 cyc/row on the PE and halves HBM traffic.
  - main matmuls use the RAW weights (depend only on DMAs); both cosine
    normalizers are applied at output time in one fused DVE op:
        out[c,b] = (logits[c,b] * rwT[c]) * f[b]
    where rwT[c] = scale_k/||w_kc|| enters as the per-partition scalar of
    scalar_tensor_tensor and f[b] = 1/||x_bk|| is row-broadcast to a
    [100,512] SBUF tile by a gpsimd (SWDGE) DMA.
  - row sums-of-squares go through selector matmuls on the PE (contraction
    over d must sit on partitions), accumulated in PSUM; squares are split
    between ACT and DVE; reciprocal on DVE, sqrt on ACT.
  - rw is computed in [k,c] layout (k on partitions so scale^2 folds into
    the sqrt as a per-partition scale) and PE-transposed to [c,k].
  - small/const DMAs ride the gpsimd SWDGE queue so the sync HWDGE queue
    streams w then x back-to-back from t=0.
"""

import time

import numpy as np
import ml_dtypes

import concourse.bass as bass
import concourse.mybir as mybir
import concourse.tile as tile
from concourse import bacc
from concourse.bass_utils import run_bass_kernel_spmd

N_CORES = 8
B, K, D, C, E = 4096, 8, 1024, 100, 16
BLOC = B // N_CORES  # 512
P = 128
DC = D // P  # 8 d-chunks
HALF = 4  # k-batch size for the x-norm pipeline
JA = 5  # x^2 d-chunks on ACT (rest on DVE)
JB = DC - JA

F32 = mybir.dt.float32
F32R = mybir.dt.float32r
BF16 = mybir.dt.bfloat16
AF = mybir.ActivationFunctionType
MULT = mybir.AluOpType.mult
NPBF16 = ml_dtypes.bfloat16

_CACHE = {}

LAST_RESULT = None
LAST_WALL_NS = None


def _build():
    nc = bacc.Bacc(
        "TRN2", target_bir_lowering=False, debug=False, num_devices=N_CORES
    )

    xt = nc.dram_tensor("xt", [K, P, DC, BLOC], BF16, kind="ExternalInput").ap()
    wt = nc.dram_tensor("wt", [K, P, DC, C], BF16, kind="ExternalInput").ap()
    lsg = nc.dram_tensor("lsg", [K, 1], F32, kind="ExternalInput").ap()
    selk4 = nc.dram_tensor("selk4", [P, HALF, HALF], BF16, kind="ExternalInput").ap()
    selk8 = nc.dram_tensor("selk8", [P, K, K], BF16, kind="ExternalInput").ap()
    eye8 = nc.dram_tensor("eye8", [K, K], F32, kind="ExternalInput").ap()
    selc4 = nc.dram_tensor("selc4", [HALF, HALF, C], F32R, kind="ExternalInput").ap()
    out = nc.dram_tensor("out", [K, C, BLOC], F32, kind="ExternalOutput").ap()

    with tile.TileContext(nc) as tc:
        with (
            tc.tile_pool(name="const", bufs=1) as cpool,
            tc.tile_pool(name="xres", bufs=K) as xpool,
            tc.tile_pool(name="wres", bufs=K) as wpool,
            tc.tile_pool(name="x2a", bufs=K) as x2apool,
            tc.tile_pool(name="x2b", bufs=K) as x2bpool,
            tc.tile_pool(name="small", bufs=1) as spool,
            tc.tile_pool(name="osb", bufs=3) as opool,
            tc.tile_pool(name="fb", bufs=2) as fbpool,
            tc.tile_pool(name="fx", bufs=2) as fxpool,
        ):
            # ---- small/const DMAs on the SWDGE queue ----
            selk4_sb = cpool.tile([P, HALF, HALF], BF16)
            nc.gpsimd.dma_start(selk4_sb[:], selk4[:])
            selk8_sb = cpool.tile([P, K, K], BF16)
            nc.gpsimd.dma_start(selk8_sb[:], selk8[:])
            eye8_sb = cpool.tile([K, K], F32)
            nc.gpsimd.dma_start(eye8_sb[:], eye8[:])
            selc4_sb = cpool.tile([HALF, HALF, C], F32R)
            nc.gpsimd.dma_start(selc4_sb[:], selc4[:])
            lsg_sb = cpool.tile([K, 1], F32)
            nc.gpsimd.dma_start(lsg_sb[:], lsg[:])

            # ---- bulk DMAs on the HWDGE queue: w first, then x ----
            w_sbs = []
            for k in range(K):
                w1 = wpool.tile([P, DC, C], BF16, tag="w", name=f"w{k}")
                nc.sync.dma_start(w1[:], wt[k])
                w_sbs.append(w1)
            x_sbs = []
            for k in range(K):
                x_sb = xpool.tile([P, DC, BLOC], BF16, tag="x", name=f"x{k}")
                nc.sync.dma_start(x_sb[:], xt[k])
                x_sbs.append(x_sb)

            # ---- per-expert scale: s2 = min(exp(ls), 100)^2, [K,1] ----
            s_sb = spool.tile([K, 1], F32)
            nc.scalar.activation(s_sb[:], lsg_sb[:], AF.Exp)
            nc.vector.tensor_scalar_min(s_sb[:], s_sb[:], 100.0)
            s2_sb = spool.tile([K, 1], F32)
            nc.vector.tensor_mul(s2_sb[:], s_sb[:], s_sb[:])

            # ---- W branch: wss[k,c] -> rw[k,c]=scale/||w|| -> rwT[c,k] ----
            wpsum_ctx = (
                tc.tile_pool(name="pwss", bufs=1, space="PSUM"),
                tc.tile_pool(name="pwt", bufs=1, space="PSUM"),
            )
            pwss, pwt = wpsum_ctx[0].__enter__(), wpsum_ctx[1].__enter__()
            wss = pwss.tile([K, C], F32)
            w2_sbs = []
            for k in range(K):
                w2_sb = wpool.tile([P, DC, C], BF16, tag="w2", name=f"w2_{k}")
                nc.vector.tensor_mul(w2_sb[:], w_sbs[k][:], w_sbs[k][:])
                w2_sbs.append(w2_sb)
            for k in range(K):
                for j in range(DC):
                    nc.tensor.matmul(
                        wss[:],
                        lhsT=selk8_sb[:, k, :],
                        rhs=w2_sbs[k][:, j],
                        start=(k == 0 and j == 0),
                        stop=(k == K - 1 and j == DC - 1),
                        skip_group_check=True,
                    )
            recw_sb = spool.tile([K, C], F32)
            nc.vector.reciprocal(recw_sb[:], wss[:])
            rw_sb = spool.tile([K, C], F32)
            # rw = sqrt(s2 / ||w||^2) = scale / ||w||
            nc.scalar.activation(rw_sb[:], recw_sb[:], AF.Sqrt, scale=s2_sb[:])
            rwt_ps = pwt.tile([C, K], F32)
            nc.tensor.transpose(rwt_ps[:], rw_sb[:], eye8_sb[:])
            rwt_sb = spool.tile([C, K], F32)
            nc.scalar.activation(rwt_sb[:], rwt_ps[:], AF.Copy)
            wpsum_ctx[1].__exit__(None, None, None)
            wpsum_ctx[0].__exit__(None, None, None)

            xpsum_ctx = (
                tc.tile_pool(name="pss", bufs=2, space="PSUM"),
                tc.tile_pool(name="plog", bufs=4, space="PSUM"),
                tc.tile_pool(name="pf", bufs=2, space="PSUM"),
            )
            pss = xpsum_ctx[0].__enter__()
            plog = xpsum_ctx[1].__enter__()
            pf = xpsum_ctx[2].__enter__()

            # ---- squares for all k (ACT low j-chunks, DVE high) ----
            x2as, x2bs = [], []
            for k in range(K):
                x2a = x2apool.tile([P, JA, BLOC], BF16, tag="x2a", name=f"x2a{k}")
                nc.scalar.activation(x2a[:], x_sbs[k][:, :JA], AF.Square)
                x2as.append(x2a)
            for k in range(K):
                x2b = x2bpool.tile([P, JB, BLOC], BF16, tag="x2b", name=f"x2b{k}")
                nc.vector.tensor_mul(x2b[:], x_sbs[k][:, JA:], x_sbs[k][:, JA:])
                x2bs.append(x2b)

            # ---- X pipeline, two k-halves ----
            for half in range(2):
                ss = pss.tile([HALF, BLOC], F32)
                for i in range(HALF):
                    k = half * HALF + i
                    for j in range(DC):
                        src = (
                            x2as[k][:, j] if j < JA else x2bs[k][:, j - JA]
                        )
                        nc.tensor.matmul(
                            ss[:],
                            lhsT=selk4_sb[:, i, :],
                            rhs=src,
                            start=(i == 0 and j == 0),
                            stop=(i == HALF - 1 and j == DC - 1),
                            skip_group_check=True,
                        )
                recx_sb = fxpool.tile([HALF, BLOC], F32, tag="recx")
                nc.vector.reciprocal(recx_sb[:], ss[:])
                fx_sb = fxpool.tile([HALF, BLOC], F32R, tag="fx")
                nc.scalar.activation(fx_sb[:], recx_sb[:], AF.Sqrt)

                # f-row broadcast to [C, BLOC]: selector matmul + ACT copy
                fbs = []
                for i in range(HALF):
                    fb_ps = pf.tile([C, BLOC], F32, tag="fbps", name=f"fbps{i}")
                    nc.tensor.matmul(
                        fb_ps[:],
                        lhsT=selc4_sb[:, i, :],
                        rhs=fx_sb[:],
                        start=True, stop=True,
                        skip_group_check=True,
                    )
                    fb_sb = fbpool.tile([C, BLOC], F32, tag="fb", name=f"fb{i}")
                    nc.scalar.activation(fb_sb[:], fb_ps[:], AF.Copy)
                    fbs.append(fb_sb)

                lgs = []
                for i in range(HALF):
                    k = half * HALF + i
                    lg = plog.tile([C, BLOC], F32, tag="lg", name=f"lg{k}")
                    for j in range(DC):
                        nc.tensor.matmul(
                            lg[:],
                            lhsT=w_sbs[k][:, j, :],
                            rhs=x_sbs[k][:, j],
                            start=(j == 0),
                            stop=(j == DC - 1),
                            skip_group_check=True,
                        )
                    lgs.append(lg)

                for i in range(HALF):
                    k = half * HALF + i
                    o_sb = opool.tile([C, BLOC], F32, tag="osb", name=f"o{k}")
                    # out = (logits * rwT[:,k]) * f_bcast  — one fused DVE op
                    nc.vector.scalar_tensor_tensor(
                        o_sb[:],
                        lgs[i][:],
                        rwt_sb[:, k : k + 1],
                        fbs[i][:],
                        op0=MULT,
                        op1=MULT,
                    )
                    nc.sync.dma_start(out[k], o_sb[:])

            for c in reversed(xpsum_ctx):
                c.__exit__(None, None, None)

    nc.compile()
    return nc


def _host_prep(pooled, active_expert_indices, weights, log_scales):
    idx = np.asarray(active_expert_indices).astype(np.int64)
    pooled = np.asarray(pooled, dtype=np.float32)
    weights = np.asarray(weights, dtype=np.float32)
    log_scales = np.asarray(log_scales, dtype=np.float32)

    # x: [B,K,D] -> bf16 -> per-core [K, P, DC, BLOC]  (k, d, j, b)
    pb = pooled.astype(NPBF16)
    xt_all = np.ascontiguousarray(
        pb.reshape(N_CORES, BLOC, K, DC, P).transpose(0, 2, 4, 3, 1)
    )
    # w: gather -> bf16 -> [K, P, DC, C]  (k, d, j, c)
    wg = weights[idx].astype(NPBF16)
    wt = np.ascontiguousarray(wg.reshape(K, C, DC, P).transpose(0, 3, 2, 1))
    lsg = np.ascontiguousarray(log_scales[idx].reshape(K, 1))

    selk4 = np.zeros((P, HALF, HALF), NPBF16)
    for i in range(HALF):
        selk4[:, i, i] = 1.0
    selk8 = np.zeros((P, K, K), NPBF16)
    for k in range(K):
        selk8[:, k, k] = 1.0
    eye8 = np.eye(K, dtype=np.float32)
    selc4 = np.zeros((HALF, HALF, C), np.float32)
    for i in range(HALF):
        selc4[i, i, :] = 1.0

    shared = {
        "wt": wt, "lsg": lsg, "selk4": selk4, "selk8": selk8, "eye8": eye8,
        "selc4": selc4,
    }
    return [dict(shared, xt=np.ascontiguousarray(xt_all[co]))
            for co in range(N_CORES)]


def kernel(pooled, active_expert_indices, weights, log_scales):
    global LAST_RESULT, LAST_WALL_NS
    if "nc" not in _CACHE:
        _CACHE["nc"] = _build()
    nc = _CACHE["nc"]

    in_maps = _host_prep(pooled, active_expert_indices, weights, log_scales)

    t0 = time.perf_counter_ns()
    res = run_bass_kernel_spmd(nc, in_maps, core_ids=list(range(N_CORES)))
    LAST_WALL_NS = time.perf_counter_ns() - t0
    LAST_RESULT = res

    full = np.stack([res.results[co]["out"] for co in range(N_CORES)])
    return np.ascontiguousarray(
        full.transpose(0, 3, 1, 2).reshape(B, K, C)
    ).astype(np.float32)


# revision 22
# speedup vs baseline: 49439.5154x; 1.1382x over previous
"""Trainium2 Bass kernel for nn_ExpertClassifierBank.

Computes, for pooled [B,K,D], expert weights [E,C,D], indices [K], log_scales [E]:
    x = l2norm(pooled, axis=-1)
    w = l2norm(weights[idx], axis=-1)
    out[b,k,c] = min(exp(log_scales[idx[k]]), 100) * dot(x[b,k], w[k,c])

Sharding: data-parallel over batch B across 8 NeuronCores (512 rows each);
the gathered expert weight bank is replicated.

Device algorithm per core (B_loc=512, K=8, D=1024=8x128, C=100):
  - host pre-transposes x and w to [k, d, j, *] bf16 tiles so the contraction
    dim d sits on SBUF partitions (PE contracts along partitions); bf16
    streams at 1 cyc/row on the PE and halves HBM traffic.
  - main matmuls use the RAW weights (depend only on DMAs); both cosine
    normalizers are applied at output time in one fused DVE op:
        out[c,b] = (logits[c,b] * rwT[c]) * f[b]
    where rwT[c] = scale_k/||w_kc|| enters as the per-partition scalar of
    scalar_tensor_tensor and f[b] = 1/||x_bk|| is row-broadcast to a
    [100,512] SBUF tile by a gpsimd (SWDGE) DMA.
  - row sums-of-squares go through selector matmuls on the PE (contraction
    over d must sit on partitions), accumulated in PSUM; squares are split
    between ACT and DVE; reciprocal on DVE, sqrt on ACT.
  - rw is computed in [k,c] layout (k on partitions so scale^2 folds into
    the sqrt as a per-partition scale) and PE-transposed to [c,k].
  - small/const DMAs ride the gpsimd SWDGE queue so the sync HWDGE queue
    streams w then x back-to-back from t=0.
"""

import time

import numpy as np
import ml_dtypes

import concourse.bass as bass
import concourse.mybir as mybir
import concourse.tile as tile
from concourse import bacc
from concourse.bass_utils import run_bass_kernel_spmd

N_CORES = 8
B, K, D, C, E = 4096, 8, 1024, 100, 16
BLOC = B // N_CORES  # 512
P = 128
DC = D // P  # 8 d-chunks
HALF = 4  # k-batch size for the x-norm pipeline
JA = 5  # x^2 d-chunks on ACT (rest on DVE)
JB = DC - JA

F32 = mybir.dt.float32
F32R = mybir.dt.float32r
BF16 = mybir.dt.bfloat16
AF = mybir.ActivationFunctionType
MULT = mybir.AluOpType.mult
NPBF16 = ml_dtypes.bfloat16

_CACHE = {}

LAST_RESULT = None
LAST_WALL_NS = None


def _build():
    nc = bacc.Bacc(
        "TRN2", target_bir_lowering=False, debug=False, num_devices=N_CORES
    )

    xt = nc.dram_tensor("xt", [K, P, DC, BLOC], BF16, kind="ExternalInput").ap()
    wt = nc.dram_tensor("wt", [K, P, DC, C], BF16, kind="ExternalInput").ap()
    lsg = nc.dram_tensor("lsg", [K, 1], F32, kind="ExternalInput").ap()
    selk4 = nc.dram_tensor("selk4", [P, HALF, HALF], BF16, kind="ExternalInput").ap()
    selk8 = nc.dram_tensor("selk8", [P, K, K], BF16, kind="ExternalInput").ap()
    eye8 = nc.dram_tensor("eye8", [K, K], F32, kind="ExternalInput").ap()
    selc4 = nc.dram_tensor("selc4", [HALF, HALF, C], F32R, kind="ExternalInput").ap()
    out = nc.dram_tensor("out", [K, C, BLOC], F32, kind="ExternalOutput").ap()

    with tile.TileContext(nc) as tc:
        with (
            tc.tile_pool(name="const", bufs=1) as cpool,
            tc.tile_pool(name="xres", bufs=K) as xpool,
            tc.tile_pool(name="wres", bufs=K) as wpool,
            tc.tile_pool(name="x2a", bufs=K) as x2apool,
            tc.tile_pool(name="x2b", bufs=K) as x2bpool,
            tc.tile_pool(name="small", bufs=1) as spool,
            tc.tile_pool(name="osb", bufs=5) as opool,
            tc.tile_pool(name="fb", bufs=2) as fbpool,
            tc.tile_pool(name="fx", bufs=2) as fxpool,
        ):
            # ---- small/const DMAs on the SWDGE queue ----
            selk4_sb = cpool.tile([P, HALF, HALF], BF16)
            nc.gpsimd.dma_start(selk4_sb[:], selk4[:])
            selk8_sb = cpool.tile([P, K, K], BF16)
            nc.gpsimd.dma_start(selk8_sb[:], selk8[:])
            eye8_sb = cpool.tile([K, K], F32)
            nc.gpsimd.dma_start(eye8_sb[:], eye8[:])
            selc4_sb = cpool.tile([HALF, HALF, C], F32R)
            nc.gpsimd.dma_start(selc4_sb[:], selc4[:])
            lsg_sb = cpool.tile([K, 1], F32)
            nc.gpsimd.dma_start(lsg_sb[:], lsg[:])

            # ---- bulk DMAs on the HWDGE queue: w first, then x ----
            w_sbs = []
            for k in range(K):
                w1 = wpool.tile([P, DC, C], BF16, tag="w", name=f"w{k}")
                nc.sync.dma_start(w1[:], wt[k])
                w_sbs.append(w1)
            x_sbs = []
            for k in range(K):
                x_sb = xpool.tile([P, DC, BLOC], BF16, tag="x", name=f"x{k}")
                nc.sync.dma_start(x_sb[:], xt[k])
                x_sbs.append(x_sb)

            # ---- per-expert scale: s2 = min(exp(ls), 100)^2, [K,1] ----
            s_sb = spool.tile([K, 1], F32)
            nc.scalar.activation(s_sb[:], lsg_sb[:], AF.Exp)
            nc.vector.tensor_scalar_min(s_sb[:], s_sb[:], 100.0)
            s2_sb = spool.tile([K, 1], F32)
            nc.vector.tensor_mul(s2_sb[:], s_sb[:], s_sb[:])

            # ---- W branch: wss[k,c] -> rw[k,c]=scale/||w|| -> rwT[c,k] ----
            wpsum_ctx = (
                tc.tile_pool(name="pwss", bufs=1, space="PSUM"),
                tc.tile_pool(name="pwt", bufs=1, space="PSUM"),
            )
            pwss, pwt = wpsum_ctx[0].__enter__(), wpsum_ctx[1].__enter__()
            wss = pwss.tile([K, C], F32)
            w2_sbs = []
            for k in range(K):
                w2_sb = wpool.tile([P, DC, C], BF16, tag="w2", name=f"w2_{k}")
                nc.vector.tensor_mul(w2_sb[:], w_sbs[k][:], w_sbs[k][:])
                w2_sbs.append(w2_sb)
            for k in range(K):
                for j in range(DC):
                    nc.tensor.matmul(
                        wss[:],
                        lhsT=selk8_sb[:, k, :],
                        rhs=w2_sbs[k][:, j],
                        start=(k == 0 and j == 0),
                        stop=(k == K - 1 and j == DC - 1),
                        skip_group_check=True,
                    )
            recw_sb = spool.tile([K, C], F32)
            nc.vector.reciprocal(recw_sb[:], wss[:])
            rw_sb = spool.tile([K, C], F32)
            # rw = sqrt(s2 / ||w||^2) = scale / ||w||
            nc.scalar.activation(rw_sb[:], recw_sb[:], AF.Sqrt, scale=s2_sb[:])
            rwt_ps = pwt.tile([C, K], F32)
            nc.tensor.transpose(rwt_ps[:], rw_sb[:], eye8_sb[:])
            rwt_sb = spool.tile([C, K], F32)
            nc.scalar.activation(rwt_sb[:], rwt_ps[:], AF.Copy)
            wpsum_ctx[1].__exit__(None, None, None)
            wpsum_ctx[0].__exit__(None, None, None)

            xpsum_ctx = (
                tc.tile_pool(name="pss", bufs=2, space="PSUM"),
                tc.tile_pool(name="plog", bufs=2, space="PSUM"),
                tc.tile_pool(name="pf", bufs=4, space="PSUM"),
            )
            pss = xpsum_ctx[0].__enter__()
            plog = xpsum_ctx[1].__enter__()
            pf = xpsum_ctx[2].__enter__()

            # ---- X pipeline: per-k interleaved ss+main matmuls ----
            # trace order is engine-queue order (queues are in-order); the
            # f-chain for each half is traced at its gating k (3 and 7)
            sss = []
            recx_sbs = []
            fx_sbs = []
            fbs = {}
            lgs_sbs = {}
            for k in range(K):
                half, i = divmod(k, HALF)
                if i == 0:
                    ss = pss.tile([HALF, BLOC], F32, tag="ss", name=f"ss{half}")
                    sss.append(ss)
                ss = sss[half]
                x2a = x2apool.tile([P, JA, BLOC], BF16, tag="x2a", name=f"x2a{k}")
                nc.scalar.activation(x2a[:], x_sbs[k][:, :JA], AF.Square)
                x2b = x2bpool.tile([P, JB, BLOC], BF16, tag="x2b", name=f"x2b{k}")
                nc.vector.tensor_mul(x2b[:], x_sbs[k][:, JA:], x_sbs[k][:, JA:])
                for j in range(DC):
                    src_ap = x2a[:, j] if j < JA else x2b[:, j - JA]
                    nc.tensor.matmul(
                        ss[:],
                        lhsT=selk4_sb[:, i, :],
                        rhs=src_ap,
                        start=(i == 0 and j == 0),
                        stop=(i == HALF - 1 and j == DC - 1),
                        skip_group_check=True,
                    )
                if i == HALF - 1:
                    # half's ss complete -> f-chain (recip on DVE, sqrt on ACT)
                    recx_sb = fxpool.tile([HALF, BLOC], F32, tag="recx",
                                          name=f"recx{half}")
                    scr = fxpool.tile([HALF, BLOC], F32, tag="rscr",
                                      name=f"rscr{half}")
                    nc.vector.reciprocal_approx_accurate(
                        recx_sb[:], ss[:], scr[:]
                    )
                    recx_sbs.append(recx_sb)
                    fx_sb = fxpool.tile([HALF, BLOC], F32R, tag="fx",
                                        name=f"fx{half}")
                    nc.scalar.activation(fx_sb[:], recx_sb[:], AF.Sqrt)
                    fx_sbs.append(fx_sb)
                lg = plog.tile([C, BLOC], F32, tag="lg", name=f"lg{k}")
                for j in range(DC):
                    nc.tensor.matmul(
                        lg[:],
                        lhsT=w_sbs[k][:, j, :],
                        rhs=x_sbs[k][:, j],
                        start=(j == 0),
                        stop=(j == DC - 1),
                        skip_group_check=True,
                    )
                if i == HALF - 1:
                    # f-broadcast matmuls for this half (fx ready by now)
                    for ii in range(HALF):
                        kk = half * HALF + ii
                        fb_ps = pf.tile([C, BLOC], F32, tag="fbps",
                                        name=f"fbps{kk}")
                        nc.tensor.matmul(
                            fb_ps[:],
                            lhsT=selc4_sb[:, ii, :],
                            rhs=fx_sbs[half][:],
                            start=True, stop=True,
                            skip_group_check=True,
                        )
                        fbs[kk] = fb_ps
                # apply the W-normalizer while copying logits out of PSUM
                lgs_sb = opool.tile([C, BLOC], F32, tag="lgs", name=f"lgs{k}")
                nc.scalar.activation(
                    lgs_sb[:], lg[:], AF.Copy, scale=rwt_sb[:, k : k + 1]
                )
                lgs_sbs[k] = lgs_sb
                if i == HALF - 1:
                    for ii in range(HALF):
                        kk = half * HALF + ii
                        o_sb = opool.tile([C, BLOC], F32, tag="osb",
                                          name=f"o{kk}")
                        nc.vector.tensor_mul(
                            o_sb[:], lgs_sbs[kk][:], fbs[kk][:]
                        )
                        nc.sync.dma_start(out[kk], o_sb[:])

            for c in reversed(xpsum_ctx):
                c.__exit__(None, None, None)

    nc.compile()
    return nc


def _host_prep(pooled, active_expert_indices, weights, log_scales):
    idx = np.asarray(active_expert_indices).astype(np.int64)
    pooled = np.asarray(pooled, dtype=np.float32)
    weights = np.asarray(weights, dtype=np.float32)
    log_scales = np.asarray(log_scales, dtype=np.float32)

    # x: [B,K,D] -> bf16 -> per-core [K, P, DC, BLOC]  (k, d, j, b)
    pb = pooled.astype(NPBF16)
    xt_all = np.ascontiguousarray(
        pb.reshape(N_CORES, BLOC, K, DC, P).transpose(0, 2, 4, 3, 1)
    )
    # w: gather -> bf16 -> [K, P, DC, C]  (k, d, j, c)
    wg = weights[idx].astype(NPBF16)
    wt = np.ascontiguousarray(wg.reshape(K, C, DC, P).transpose(0, 3, 2, 1))
    lsg = np.ascontiguousarray(log_scales[idx].reshape(K, 1))

    selk4 = np.zeros((P, HALF, HALF), NPBF16)
    for i in range(HALF):
        selk4[:, i, i] = 1.0
    selk8 = np.zeros((P, K, K), NPBF16)
    for k in range(K):
        selk8[:, k, k] = 1.0
    eye8 = np.eye(K, dtype=np.float32)
    selc4 = np.zeros((HALF, HALF, C), np.float32)
    for i in range(HALF):
        selc4[i, i, :] = 1.0

    shared = {
        "wt": wt, "lsg": lsg, "selk4": selk4, "selk8": selk8, "eye8": eye8,
        "selc4": selc4,
    }
    return [dict(shared, xt=np.ascontiguousarray(xt_all[co]))
            for co in range(N_CORES)]


def kernel(pooled, active_expert_indices, weights, log_scales):
    global LAST_RESULT, LAST_WALL_NS
    if "nc" not in _CACHE:
        _CACHE["nc"] = _build()
    nc = _CACHE["nc"]

    in_maps = _host_prep(pooled, active_expert_indices, weights, log_scales)

    t0 = time.perf_counter_ns()
    res = run_bass_kernel_spmd(nc, in_maps, core_ids=list(range(N_CORES)))
    LAST_WALL_NS = time.perf_counter_ns() - t0
    LAST_RESULT = res

    full = np.stack([res.results[co]["out"] for co in range(N_CORES)])
    return np.ascontiguousarray(
        full.transpose(0, 3, 1, 2).reshape(B, K, C)
    ).astype(np.float32)
